# revision 6
# baseline (speedup 1.0000x reference)
"""Trainium2 Bass kernel for nn_CustomConv2d: 3x3 conv, stride 1, pad 1.

Full shapes: x (32,128,56,56) f32, weight (256,128,3,3) f32, bias (256,) f32.
Output: (32,256,56,56) f32.

Strategy: data-parallel over batch (8 cores x 4 images). Per image the conv is
9 accumulating PE matmuls per output tile: contraction dim = Cin = 128 (the
full PE array), stationary = weight tap (Cin x 128-cout-half), moving = a
shifted window of the zero-padded input, free dim = rows x 56 cols. The PE
stream is packed back-to-back (~94.6us of the ~100.8us total = the 1 col/cycle
algorithmic floor at 2.4GHz).

Key choices:
- x and weight stream as bf16 (halves input DMA; adds ~2e-3 max rel err, well
  under the 2e-2 gate). PSUM accumulation stays f32; output is exact f32.
- Stores run through SWDGE kv_writeback prepare/trigger: one prep per
  (image, cout-half) writes all 14 x 224-f32 pieces of a [128, 3136]
  half-image in a single 113-descriptor DMA. Desc-gen runs ahead on the idle
  GpSimd engine; the trigger fires right after the last PSUM->SBUF copy, so
  the tail skips the HWDGE gen + DGE->DMA handoff latency entirely.
- The preps' data-input RAW edges are demoted to their triggers (the same
  deferral the framework applies to gather/scatter preps) so desc-gen never
  waits on compute.
- Tail: the final half-image ends in 7/2/2-row chunks; the 14-piece store
  fires after the rows 0-51 copies (its piece 13 is stale by design) and a
  9-descriptor store of piece 13 (rows 52-55) overwrites it, ordered by the
  shared SWDGE queue FIFO.
- Head: DMA order puts the first row slab + first 4 weight taps in the first
  two (HWDGE-serialized) transfers; one early dummy matmul starts the PE
  p-state ramp so real matmuls run at full clock from the start.
"""

import numpy as np

import concourse.bass as bass
import concourse.mybir as mybir
import concourse.tile as tile
from concourse import bacc
from concourse.bass_utils import run_bass_kernel_spmd

N_CORES = 8
B = 32
B_LOC = B // N_CORES  # 4
CIN = 128
COUT = 256
H = W = 56
HP = WP = 58  # padded
HW = H * W  # 3136 = 14 * 224

N_WARM = 1
ROWS_MAIN = [8, 8, 8, 8, 8, 8, 8]
ROWS_LAST = [9, 9, 9, 9, 9, 7, 2, 2]

_NC_CACHE = None
LAST_RESULTS = None  # stashed BassKernelResults for test harness introspection


def _kv_prep(nc, out_ap, in_ap, ctx_idxs_ap, queue_num: int = 0):
    """kv_writeback(prepare_only=True) without the user-sem then_inc: Tile's
    sem pass installs its DMASW lane sem as on_update[0] (the DMA-completion
    slot the interp/cost model defer to trigger time), so downstream waits and
    the kernel-exit barrier resolve against the lane sem."""
    from concourse.bass import exact_div
    from concourse import mybir as mb

    g = nc.gpsimd
    batch, d_head_inner, d_head_outer, n_ctx = out_ap.shape
    d_head = d_head_outer * d_head_inner
    ncn = in_ap.shape[3]
    batch_step = exact_div(in_ap.ap[1][0], ncn)
    assert in_ap.shape[2] == batch
    dtype_size = mb.dt.size(out_ap.dtype)
    assert out_ap.ap[3][0] == 1
    assert out_ap.ap[1][0] == d_head_outer * out_ap.ap[2][0]
    assert 0 < ncn < 256 and d_head % 128 == 0
    inst = g.add_instruction(
        mb.InstKVWritebackAnt(
            name=nc.get_next_instruction_name(),
            ins=[g.lower_ap(in_ap), g.lower_ap(ctx_idxs_ap)],
            outs=[*g.lower_ap_dma(out_ap.opt([0]), for_custom_bir_dma=True)],
            batch=batch,
            batch_step=batch_step,
            ncn=0,
            ncn_raw=ncn,
            d_head=exact_div(d_head, 128),
            wraparound=False,
            n_ctx=n_ctx,
            gen_mode=1,
            dho_stride_bytes=out_ap.ap[2][0] * dtype_size,
            batch_stride_bytes=out_ap.ap[0][0] * dtype_size,
            queue_num=queue_num,
        )
    )
    return g._track_prepare_only(inst, queue_num)


def _build(reps: int = 1) -> bass.Bass:
    f32 = mybir.dt.float32
    bf16 = mybir.dt.bfloat16
    i32 = mybir.dt.int32
    nc = bacc.Bacc(None, target_bir_lowering=False)
    x_d = nc.dram_tensor("x", [B_LOC, CIN, HP * WP], bf16, kind="ExternalInput")
    wt_d = nc.dram_tensor("wt", [2, CIN, 9 * 128], bf16, kind="ExternalInput")
    b_d = nc.dram_tensor("b", [2, 128], f32, kind="ExternalInput")
    y_d = nc.dram_tensor("y", [B_LOC, COUT, H * W], f32, kind="ExternalOutput")


    from contextlib import ExitStack, nullcontext

    with tile.TileContext(nc) as tc, ExitStack() as es:
        cpool = es.enter_context(tc.tile_pool(name="const", bufs=1))
        xpool = es.enter_context(tc.tile_pool(name="xp", bufs=B_LOC))
        opool = es.enter_context(tc.tile_pool(name="out", bufs=3))
        pspool = es.enter_context(tc.tile_pool(name="ps", bufs=7, space="PSUM"))
        with tc.For_i(0, reps, 1) if reps > 1 else nullcontext():
            # wtile[:, t, tap, :]: stationary for (cout-half t, tap)
            wtile = cpool.tile([CIN, 2, 9, 128], bf16)
            xpads = [
                xpool.tile([CIN, HP, WP], bf16, tag="xpad", name=f"xpad{i}")
                for i in range(B_LOC)
            ]
            btile = cpool.tile([128, 2], f32)
            zidx = cpool.tile([128, 1], i32)
            cidx2 = cpool.tile([128, 1], i32)

            # PE warmup: dep-free dummy matmuls fill the initial DMA wait and
            # bring the PE clock (HAM) to full rate before the real work.
            wsrc0 = cpool.tile([128, 64], f32)
            nc.gpsimd.memset(wsrc0[:], 0.0)
            nc.gpsimd.memset(zidx[:], 0)
            nc.gpsimd.memset(cidx2[:], (14 - 1) * 224)
            wsrc = wsrc0[:].bitcast(bf16)[:, 0:64]
            wps = pspool.tile([64, 64], f32, tag="warmps", bufs=1)
            for _ in range(N_WARM):
                nc.tensor.matmul(wps[:], wsrc, wsrc, start=True, stop=True)

            # DMA issue order = criticality: t0 tap weights first (each tap is
            # consumed 187ns after the previous), then the row-0 slab, then the
            # rest paced ahead of consumption.
            xsrc0 = x_d[0].rearrange("p (h w) -> p h w", h=HP)
            wt0 = wt_d[0].rearrange("p (t o) -> p t o", t=9)
            wt1 = wt_d[1].rearrange("p (t o) -> p t o", t=9)
            nc.sync.dma_start(xpads[0][:, 0:10, :], xsrc0[:, 0:10, :])
            nc.sync.dma_start(wtile[:, 0, 0:4, :], wt0[:, 0:4, :])
            nc.sync.dma_start(wtile[:, 0, 4:9, :], wt0[:, 4:9, :])
            nc.sync.dma_start(xpads[0][:, 10:18, :], xsrc0[:, 10:18, :])
            nc.sync.dma_start(xpads[0][:, 18:34, :], xsrc0[:, 18:34, :])
            nc.sync.dma_start(xpads[0][:, 34:58, :], xsrc0[:, 34:58, :])
            nc.sync.dma_start(wtile[:, 1, :, :], wt1[:, :, :])
            nc.sync.dma_start(btile[:], b_d[:].rearrange("t p -> p t"))
            for b in range(1, B_LOC):
                xsrc = x_d[b].rearrange("p (h w) -> p h w", h=HP)
                nc.sync.dma_start(xpads[b][:, 0:29, :], xsrc[:, 0:29, :])
                nc.sync.dma_start(xpads[b][:, 29:58, :], xsrc[:, 29:58, :])

            import bass_rust as _br

            DI = mybir.DependencyInfo
            for b in range(B_LOC):
                xpad = xpads[b]
                for t in range(2):
                    last_bt = (b == B_LOC - 1) and (t == 1)
                    obuf = opool.tile([128, HW], f32, tag="obuf")
                    rows = ROWS_LAST if last_bt else ROWS_MAIN
                    r0 = 0
                    copy_names = []
                    for i, nr in enumerate(rows):
                        ps = pspool.tile([128, nr * W], f32, tag="ps")
                        psv3 = ps[:].rearrange("p (h w) -> p h w", h=nr)
                        for tap in range(9):
                            ky, kx = divmod(tap, 3)
                            rhs = xpad[:, r0 + ky : r0 + ky + nr, kx : kx + W]
                            lhsT = wtile[:, t, tap, :]
                            nc.tensor.matmul(
                                psv3, lhsT, rhs, start=(tap == 0), stop=(tap == 8)
                            )
                        dst = obuf[:, r0 * W : (r0 + nr) * W]
                        if i % 2 == 0:
                            cp = nc.scalar.activation(
                                dst,
                                ps[:],
                                mybir.ActivationFunctionType.Identity,
                                bias=btile[:, t : t + 1],
                            )
                            copy_names.append(cp.ins.name)
                        else:
                            cp = nc.vector.tensor_scalar_add(
                                dst, ps[:], btile[:, t : t + 1]
                            )
                            copy_names.append(cp.ins.name)
                        r0 += nr
                    # one SWDGE store for the whole (image, cout-half):
                    # dst[0, c, d, 0:224] = src[c, d, 0, :]  ->
                    # y[b, t*128+c, d*224 + j]
                    y_bt = y_d[b, t * 128 : (t + 1) * 128, :]

                    def _defer(prep, trigger, names):
                        # defer the prep's data-input RAW edges (obuf copies)
                        # to the trigger: desc-gen reads only metadata, the
                        # DMA reads obuf when the trigger fires (the demotion
                        # the Rust swdge_deferred_ins table applies to
                        # gather/scatter).
                        nameset = _br.InstructionNameOrderedSet(names)
                        for name in names:
                            prep.ins.try_remove_dependency(name, DI.SYNC_ONLY)
                        prep.ins.add_nosync_dependencies_from(nameset)
                        trigger.ins.add_sync_dependencies_from(nameset)

                    def _store(out4, in4, cidx, names):
                        prep = _kv_prep(nc, out4, in4, cidx)
                        trigger = nc.gpsimd.trigger_dma(count=None)
                        _defer(prep, trigger, names)
                        return prep, trigger

                    if not last_bt:
                        _store(
                            y_bt.rearrange("c (a d n) -> a c d n", a=1, d=14),
                            obuf[:].rearrange("p (d a n) -> p d a n", d=14, a=1),
                            zidx[:],
                            copy_names,
                        )
                    else:
                        # tail split: the 14-piece grid store fires after the
                        # rows 0-51 copies (piece 13 carries stale obuf bytes),
                        # then a 9-descriptor store of piece 13 (rows 52-55 =
                        # the final 4-row chunk) overwrites it. Same SWDGE
                        # queue -> FIFO-ordered on hardware.
                        # both desc-gens emitted before either trigger so
                        # neither lands behind a trigger wait on the Pool FIFO
                        prep_a = _kv_prep(
                            nc,
                            y_bt.rearrange("c (a d n) -> a c d n", a=1, d=14),
                            obuf[:].rearrange("p (d a n) -> p d a n", d=14, a=1),
                            zidx[:],
                        )
                        prep_b = _kv_prep(
                            nc,
                            y_bt.rearrange("c (a d n) -> a c d n", a=1, d=1),
                            obuf[:, 13 * 224 :].rearrange(
                                "p (d a n) -> p d a n", d=1, a=1
                            ),
                            cidx2[:],
                        )
                        # drop the WAW chain on piece 13 (A then B is already
                        # FIFO-ordered by the shared SWDGE queue); keep the
                        # no-sync edge for topological order.
                        prep_b.ins.try_remove_dependency(
                            prep_a.ins.name, DI.SYNC_ONLY
                        )
                        prep_b.ins.add_nosync_dependencies_from(
                            _br.InstructionNameOrderedSet([prep_a.ins.name])
                        )
                        trig_a = nc.gpsimd.trigger_dma(count=1)
                        trig_b = nc.gpsimd.trigger_dma(count=1)
                        # explicit-count triggers: gate each on its own prep's
                        # engine tick (desc-gen commit) plus the copies whose
                        # data its DMA reads
                        # store A must not wait the chunk-6 copy: its
                        # piece 13 bytes are stale by design (overwritten by
                        # store B, which reads rows 52-55 = chunks 5+6).
                        # Demote ALL copies from prep_a, route only rows 0-53
                        # copies to trigger A.
                        _defer(prep_a, trig_a, copy_names)
                        for name in copy_names[-2:]:
                            trig_a.ins.try_remove_dependency(name, DI.SYNC_ONLY)
                        _defer(prep_b, trig_b, copy_names[-2:])
                        # trig_a must not gate on prep_b's desc-gen tick
                        trig_a.ins.try_remove_dependency(
                            prep_b.ins.name, DI.NO_SYNC_ONLY
                        )
                        trig_a.ins.try_remove_dependency(
                            prep_b.ins.name, DI.SYNC_ONLY
                        )
                        # count=1 pops the ring FIFO oldest-first: pin the
                        # trigger order so pops match the intended preps
                        trig_b.ins.add_nosync_dependencies_from(
                            _br.InstructionNameOrderedSet([trig_a.ins.name])
                        )
    nc.finalize()
    return nc


def kernel(x, weight, bias, approximate):
    """Full (unsharded) conv2d. `approximate` only selects the HW approximation
    level in the original module; the exact-math output is independent of it."""
    global _NC_CACHE, LAST_RESULTS
    import ml_dtypes

    bf16 = ml_dtypes.bfloat16
    x = np.ascontiguousarray(x, dtype=np.float32)
    weight = np.ascontiguousarray(weight, dtype=np.float32)
    bias = np.ascontiguousarray(bias, dtype=np.float32)

    # zero-pad spatially on the host; shard batch across cores
    xp = np.zeros((B, CIN, HP, WP), bf16)
    xp[:, :, 1 : H + 1, 1 : W + 1] = x
    xp = xp.reshape(B, CIN, HP * WP)
    # wt[t, cin, tap*128 + o] = weight[t*128 + o, cin, ky, kx]
    w9 = weight.transpose(1, 2, 3, 0).reshape(CIN, 9, COUT)
    wt = np.stack([w9[:, :, 0:128], w9[:, :, 128:256]], axis=0)
    wt = np.ascontiguousarray(wt).reshape(2, CIN, 9 * 128).astype(bf16)
    b2 = bias.reshape(2, 128)

    if _NC_CACHE is None:
        _NC_CACHE = _build()
    nc = _NC_CACHE

    in_maps = [
        {"x": xp[c * B_LOC : (c + 1) * B_LOC], "wt": wt, "b": b2}
        for c in range(N_CORES)
    ]
    try:
        res = run_bass_kernel_spmd(nc, in_maps, core_ids=list(range(N_CORES)))
    except Exception:
        # transient device-acquisition races (NRT_EXEC_UNIT_UNRECOVERABLE on
        # first touch after a prior process teardown) recover on retry
        import time as _time

        _time.sleep(5.0)
        res = run_bass_kernel_spmd(nc, in_maps, core_ids=list(range(N_CORES)))
    LAST_RESULTS = res
    out = np.concatenate([r["y"] for r in res.results], axis=0)
    return out.reshape(B, COUT, H, W)
